# revision 16
# baseline (speedup 1.0000x reference)
"""Single-head attention (B=8, S=2048, E=1024, H=128) with softmax + deterministic
dropout, data-parallel over batch across 8 NeuronCores (one batch element per core).

Per-core layout strategy ("transposed attention", v6):
  - host ships x ONCE, fp16, in quad-major layout xq [4][128][NE*4*128]:
    xq[qd][p][(e*4+b)*128+ti] = x[(qd*4+b)*128+ti, e*128+p], DMA'd in half-quads.
  - ACT casts each half-quad to fp8e4m3 on-chip (the ACT engine is idle during
    the projection phase); q/k projections then run fp8 DoubleRow (2 e-chunks
    per matmul at 0.5 cycles/col) against host-shipped fp8 wq/wk (scaled by 16,
    compensated via softmax scale SCALE/256).  The v projection stays fp16 off
    the same quad tile (value-path precision).  This makes the projection phase
    DMA-paced instead of PE-paced.
  - host ships keep (dropout mask) [NSG, 128, NT*SG] fp16 {0,1} slabs, DMA'd in
    4 slices per s-group.
  - attT[t, s] pairs: two 128x512 fp16 score matmuls into one 2-bank PSUM tile;
    expT = exp(attT * SCALE/256) (ACT, 1024-wide — the attention pacing engine);
    attd = expT * keep (DVE, 1024-wide)
  - denominator: adjacent pair-sums on DVE (exp_2j + exp_2j+1), second-level
    pair-sums on GPSIMD (otherwise idle), then FOUR M=1 512-col PE matmuls per
    s-group, deferred into the next group's slots so they never starve the
    score->exp chain.  The last group keeps its level-2 sums on DVE so the
    drain stays short.
  - outT[h, s] += v[t].T @ attd  (PE fp16, fp32 PSUM accumulation)
  - outT (f16) and denom (f32) ship to DRAM unnormalized; the host computes
    out = (outT / (0.9 * den)).T  (layout choice + cheap elementwise on host)

Precision: fp8 q/k adds ~0.9% relative error through the softmax logits; the
fp16 value path contributes ~5e-4.  Total ~0.9% against the 2e-2 gate.
"""

import sys

for _p in ("/opt/trn_rl_repo",):
    if _p not in sys.path:
        sys.path.append(_p)

import numpy as np

B, S, E, H = 8, 2048, 1024, 128
DROP_P = 0.1
P = 128
W8_SCALE = 16.0

_program_cache = {}


def _build_program(S=S, E=E):
    key = (S, E)
    if key in _program_cache:
        return _program_cache[key]
    NT = S // P   # t-chunks
    NE = E // P   # e-chunks
    SG = 512      # s-group width (one fp32 PSUM bank)
    NSG = S // SG
    NQ = NT // 4  # quads of t-chunks (= s column groups)
    NPAIR = NT // 2  # pairs of t-chunks sharing a 2-bank psum tile

    import concourse.bass as bass  # noqa: F401
    import concourse.mybir as mybir
    import concourse.tile as tile
    from concourse import bacc

    f32 = mybir.dt.float32
    f16 = mybir.dt.float16
    f8 = mybir.dt.float8e4
    Exp = mybir.ActivationFunctionType.Exp
    Copy = mybir.ActivationFunctionType.Copy
    DR = mybir.MatmulPerfMode.DoubleRow
    SCALE = float(E) ** -0.5 / (W8_SCALE * W8_SCALE)

    nc = bacc.Bacc("TRN2", target_bir_lowering=False, debug=False)
    xq_d = nc.dram_tensor("xq", [NQ, P, NE * 4 * P], f16, kind="ExternalInput").ap()
    keep_d = nc.dram_tensor("keepg", [NSG, P, NT * SG], f16, kind="ExternalInput").ap()
    wall_d = nc.dram_tensor("wall", [P, 3 * NE * H], f16, kind="ExternalInput").ap()
    w8_d = nc.dram_tensor("w8", [P, 2 * NE * H], f8, kind="ExternalInput").ap()
    outT_d = nc.dram_tensor("outT", [P, S], f16, kind="ExternalOutput").ap()
    den_d = nc.dram_tensor("den", [1, S], f32, kind="ExternalOutput").ap()

    xq_r = xq_d.rearrange("q p (e b t) -> q p e b t", e=NE, b=4)
    wall_r = wall_d.rearrange("p (j eo h) -> p j eo h", j=3, eo=NE)
    w8_r = w8_d.rearrange("p (j eo h) -> p j eo h", j=2, eo=NE)
    keep_r = keep_d.rearrange("g p (t s) -> g p t s", t=NT)

    with tile.TileContext(nc) as tc:
        with (
            tc.tile_pool(name="consts", bufs=1) as consts,
            tc.tile_pool(name="xw", bufs=1) as xw_pool,
            tc.tile_pool(name="qkv", bufs=1) as qkv_pool,
            tc.tile_pool(name="keep_pool", bufs=8) as keep_pool,
        ):
            ones_t = consts.tile([P, 1], f16)
            nc.vector.memset(ones_t, 1.0)
            warm_sb = consts.tile([P, P], f16)
            nc.vector.memset(warm_sb, 0.0)

            # -------- input DMAs: one issuer (SP), priority order --------
            wall_sb = xw_pool.tile([P, 3, NE, H], f16, tag="wall", name="wall")
            wv_sb = wall_sb[:, 2]
            w8_sb = xw_pool.tile([P, 2, NE, H], f8, tag="w8", name="w8")
            nc.sync.dma_start(w8_sb, w8_r)
            nc.sync.dma_start(wall_sb, wall_r)

            # dropout mask: 4 slices per s-group, fetched on a rolling basis
            keeps = {}  # (sg, sl) -> tile [P, 4, SG]

            def fetch_keep(sg):
                for sl in range(4):
                    kt = keep_pool.tile([P, 4, SG], f16, tag=f"keep{sl}",
                                        name=f"keep{sg}_{sl}")
                    nc.sync.dma_start(kt, keep_r[sg, :, 4 * sl:4 * sl + 4, :])
                    keeps[(sg, sl)] = kt

            # -------- projections, quad-major, DMA-paced --------
            qkT_sb = qkv_pool.tile([P, 2, S], f16)  # [h, (q|k), s]
            v_sb = qkv_pool.tile([P, NT, H], f16)   # v natural: [t_in, t_chunk, h]
            with (
                tc.tile_pool(name="xq_pool", bufs=1) as xq_pool,
                tc.tile_pool(name="proj_ps", bufs=2, space="PSUM") as proj_ps,
            ):
                xq_sbs, x8_sbs = [], []
                for qd in range(NQ):
                    xq_sbs.append(xq_pool.tile([P, NE, 4, P], f16, tag="xq",
                                               name=f"xq{qd}", bufs=3))
                    x8_sbs.append(xq_pool.tile([P, NE, 4, P], f8, tag="x8",
                                               name=f"x8{qd}", bufs=2))
                EH = NE // 2
                for qd in range(NQ):
                    nc.sync.dma_start(xq_sbs[qd][:, 0:EH], xq_r[qd, :, 0:EH])
                    nc.sync.dma_start(xq_sbs[qd][:, EH:NE], xq_r[qd, :, EH:NE])
                fetch_keep(0)

                # PE clock warm-up: keep the array busy during the DMA
                # lead-in so the quad matmuls run at the ramped p-state
                ps_w = proj_ps.tile([P, P], f32, tag="warm", name="ps_warm")
                for i in range(28):
                    nc.tensor.matmul(ps_w, warm_sb, warm_sb,
                                     start=(i == 0), stop=(i == 27))

                for qd in range(NQ):
                    xqt, x8t = xq_sbs[qd], x8_sbs[qd]
                    c_sl = slice(qd * SG, (qd + 1) * SG)
                    # fp8 cast per half-quad on the (otherwise idle) ACT engine
                    nc.scalar.activation(x8t[:, 0:EH], xqt[:, 0:EH], Copy)
                    nc.scalar.activation(x8t[:, EH:NE], xqt[:, EH:NE], Copy)
                    ps_q = proj_ps.tile([P, SG], f32, tag="pq", name="ps_q")
                    ps_k = proj_ps.tile([P, SG], f32, tag="pk", name="ps_k")
                    for ep in range(NE // 2):
                        es = slice(2 * ep, 2 * ep + 2)
                        nc.tensor.matmul(
                            ps_q, w8_sb[:, 0, es, :], x8t[:, es],
                            start=(ep == 0), stop=(ep == NE // 2 - 1),
                            perf_mode=DR,
                        )
                    for ep in range(NE // 2):
                        es = slice(2 * ep, 2 * ep + 2)
                        nc.tensor.matmul(
                            ps_k, w8_sb[:, 1, es, :], x8t[:, es],
                            start=(ep == 0), stop=(ep == NE // 2 - 1),
                            perf_mode=DR,
                        )
                    nc.vector.tensor_copy(qkT_sb[:, 0, c_sl], ps_q)
                    nc.vector.tensor_copy(qkT_sb[:, 1, c_sl], ps_k)
                    for b in range(4):
                        t = 4 * qd + b
                        ps_v = proj_ps.tile([P, H], f32, tag="pv", name="ps_v")
                        for e in range(NE):
                            nc.tensor.matmul(
                                ps_v, xqt[:, e, b, :], wv_sb[:, e, :],
                                start=(e == 0), stop=(e == NE - 1),
                            )
                        nc.vector.tensor_copy(v_sb[:, t, :], ps_v)

            # -------- main attention loop over s-groups --------
            with (
                tc.tile_pool(name="att_ps", bufs=2, space="PSUM") as att_ps,
                tc.tile_pool(name="out_ps", bufs=2, space="PSUM") as out_ps,
                tc.tile_pool(name="den_ps", bufs=2, space="PSUM") as den_ps,
                tc.tile_pool(name="sb", bufs=10) as sb_pool,
                tc.tile_pool(name="sb2", bufs=2) as sb2_pool,
            ):
                # deferred denominator work from the previous s-group:
                # (psum_den, sums2_tile, s_slice)
                pending_den = [None]

                def emit_den_step(j):
                    if pending_den[0] is None or j >= 4:
                        return
                    p_den, p_sums2, p_sl = pending_den[0]
                    nc.tensor.matmul(
                        p_den, ones_t, p_sums2[:, j // 2, j % 2, :],
                        start=(j == 0), stop=(j == 3),
                    )
                    if j == 3:
                        den_sb = sb2_pool.tile([1, SG], f32, tag="den_sb")
                        nc.vector.tensor_copy(den_sb, p_den)
                        nc.gpsimd.dma_start(den_d[:, p_sl], den_sb)
                        pending_den[0] = None

                for sg in range(NSG):
                    last = sg == NSG - 1
                    s_lo = sg * SG
                    s_sl = slice(s_lo, s_lo + SG)
                    if sg + 1 < NSG:
                        fetch_keep(sg + 1)
                    keep_sls = [keeps.pop((sg, sl)) for sl in range(4)]
                    psum_out = out_ps.tile([P, SG], f32, tag="out")
                    psum_den_t = den_ps.tile([P, SG], f32, tag="den")
                    psum_den = psum_den_t[0:1, :]
                    # adjacent pair-sums (DVE), then second-level sums
                    sums = sb_pool.tile([P, 4, 2, SG], f16, tag="sums",
                                        name=f"sums{sg}", bufs=2)
                    sums2 = sb_pool.tile([P, 2, 2, SG], f16, tag="sums2",
                                         name=f"sums2_{sg}", bufs=2)
                    expTs = {}
                    attds = {}

                    def emit_front(j, s_sl=s_sl, keep_sls=keep_sls,
                                   expTs=expTs, attds=attds):
                        psum_att = att_ps.tile([P, 2, SG], f32, tag="att",
                                               name=f"att{j}")
                        for i in range(2):
                            t = 2 * j + i
                            nc.tensor.matmul(
                                psum_att[:, i, :],
                                qkT_sb[:, 1, t * P:(t + 1) * P],  # kT chunk
                                qkT_sb[:, 0, s_sl],               # qT slice
                                start=True,
                                stop=True,
                            )
                        expT = sb_pool.tile([P, 2, SG], f16, tag="exp",
                                            name=f"exp{j}", bufs=4)
                        nc.scalar.activation(expT, psum_att, Exp, scale=SCALE)
                        attd = sb_pool.tile([P, 2, SG], f16, tag="attd",
                                            name=f"attd{j}", bufs=4)
                        ksl = keep_sls[j // 2][:, (2 * j) % 4:(2 * j) % 4 + 2, :]
                        nc.vector.tensor_mul(out=attd, in0=expT, in1=ksl)
                        expTs[j] = expT
                        attds[j] = attd

                    def emit_out(j, psum_out=psum_out, attds=attds):
                        attd = attds.pop(j)
                        for i in range(2):
                            t = 2 * j + i
                            nc.tensor.matmul(
                                psum_out,
                                v_sb[:, t, :],
                                attd[:, i, :],
                                start=(t == 0),
                                stop=(t == NT - 1),
                            )

                    # software pipeline: front(j); the previous group's four
                    # deferred den matmuls on slots 0-3; adjacent pair-sums
                    # after each odd front, level-2 sums after j=3 / j=7;
                    # out(j-1)
                    for j in range(NPAIR):
                        emit_front(j)
                        emit_den_step(j)
                        if j % 2 == 1:
                            jp = j // 2
                            eng1 = (nc.gpsimd if (jp == 1 and not last)
                                    else nc.vector)
                            eng1.tensor_add(
                                out=sums[:, jp],
                                in0=expTs.pop(j - 1), in1=expTs.pop(j),
                            )
                            if jp % 2 == 1:
                                i2 = jp // 2
                                eng = nc.vector if last else nc.gpsimd
                                eng.tensor_add(
                                    out=sums2[:, i2],
                                    in0=sums[:, 2 * i2], in1=sums[:, 2 * i2 + 1],
                                )
                                if last and i2 == 0:
                                    # start the final group's den early
                                    pending_den[0] = (psum_den, sums2, s_sl)
                                    emit_den_step(0)
                                    emit_den_step(1)
                        if j >= 1:
                            emit_out(j - 1)
                    emit_out(NPAIR - 1)
                    if last:
                        emit_den_step(2)
                        emit_den_step(3)
                    else:
                        pending_den[0] = (psum_den, sums2, s_sl)

                    # unnormalized transposed output -> f16 staging -> DRAM
                    outT_sb = sb2_pool.tile([P, SG], f16, tag="outT")
                    nc.vector.tensor_copy(outT_sb, psum_out)
                    nc.gpsimd.dma_start(outT_d[:, s_sl], outT_sb)

    nc.compile()
    _program_cache[key] = nc
    return nc


def kernel(x, wq, wk, wv, drop_u):
    from concourse import bass_utils

    x = np.asarray(x)
    wq = np.asarray(wq)
    wk = np.asarray(wk)
    wv = np.asarray(wv)
    drop_u = np.asarray(drop_u)

    nc = _build_program()
    in_maps = build_in_maps(x, wq, wk, wv, drop_u)
    last_err = None
    for _attempt in range(3):
        try:
            res = bass_utils.run_bass_kernel_spmd(
                nc, in_maps, core_ids=list(range(B)), trace=False
            )
            break
        except Exception as e:  # transient device errors — retry
            last_err = e
            import time as _time

            _time.sleep(2.0)
    else:
        raise last_err
    out = np.empty((B, S, H), dtype=np.float32)
    for b in range(B):
        outT = res.results[b]["outT"].astype(np.float32)  # [H, S]
        den = np.asarray(res.results[b]["den"]).reshape(1, S).astype(np.float32)
        out[b] = (outT / ((1.0 - DROP_P) * den)).T
    return out


def _chunk_w(w, dtype, scale=1.0):
    NE = E // P
    return np.ascontiguousarray(
        (np.asarray(w) * np.float32(scale))
        .reshape(NE, P, H).transpose(1, 0, 2).reshape(P, NE * H)
    ).astype(dtype)


def build_in_maps(x, wq, wk, wv, drop_u):
    import ml_dtypes

    NT = S // P
    NE = E // P
    SG = 512
    NSG = S // SG
    NQ = NT // 4
    f8 = ml_dtypes.float8_e4m3fn
    wall = np.concatenate(
        [_chunk_w(w, np.float16) for w in (wq, wk, wv)], axis=1
    )  # [P, 3*NE*H]
    w8 = np.concatenate(
        [_chunk_w(w, f8, W8_SCALE) for w in (wq, wk)], axis=1
    )  # [P, 2*NE*H] fp8
    in_maps = []
    for b in range(B):
        # xq[qd][p][(e*4+b')*128+ti] = x[(qd*4+b')*128+ti, e*128+p]
        xq = np.ascontiguousarray(
            np.asarray(x[b]).reshape(NQ, 4, P, NE, P)  # [qd, b', ti, e, p]
            .transpose(0, 4, 3, 1, 2)                  # [qd, p, e, b', ti]
            .reshape(NQ, P, NE * 4 * P)
        ).astype(np.float16)
        keepT = (drop_u[b].T >= np.float32(DROP_P)).astype(np.float16)  # [t, s]
        keep_g = np.ascontiguousarray(
            keepT.reshape(NT, P, NSG, SG).transpose(2, 1, 0, 3).reshape(
                NSG, P, NT * SG
            )
        )
        in_maps.append({"xq": xq, "keepg": keep_g, "wall": wall, "w8": w8})
    return in_maps


# revision 18
# speedup vs baseline: 1.1550x; 1.1550x over previous
"""Single-head attention (B=8, S=2048, E=1024, H=128) with softmax + deterministic
dropout, data-parallel over batch across 8 NeuronCores (one batch element per core).

Per-core layout strategy ("transposed attention", v6):
  - host ships x ONCE, fp16, in quad-major layout xq [4][128][NE*4*128]:
    xq[qd][p][(e*4+b)*128+ti] = x[(qd*4+b)*128+ti, e*128+p], DMA'd in half-quads.
  - ACT casts each half-quad to fp8e4m3 on-chip (the ACT engine is idle during
    the projection phase); q/k projections then run fp8 DoubleRow (2 e-chunks
    per matmul at 0.5 cycles/col) against host-shipped fp8 wq/wk (scaled by 16,
    compensated via softmax scale SCALE/256).  The v projection stays fp16 off
    the same quad tile (value-path precision).  This makes the projection phase
    DMA-paced instead of PE-paced.
  - host ships keep (dropout mask) [NSG, 128, NT*SG] fp16 {0,1} slabs, DMA'd in
    4 slices per s-group.
  - attT[t, s] pairs: two 128x512 fp16 score matmuls into one 2-bank PSUM tile;
    expT = exp(attT * SCALE/256) (ACT, 1024-wide — the attention pacing engine);
    attd = expT * keep (DVE, 1024-wide)
  - denominator: adjacent pair-sums on DVE (exp_2j + exp_2j+1), second-level
    pair-sums on GPSIMD (otherwise idle), then FOUR M=1 512-col PE matmuls per
    s-group, deferred into the next group's slots so they never starve the
    score->exp chain.  The last group keeps its level-2 sums on DVE so the
    drain stays short.
  - outT[h, s] += v[t].T @ attd  (PE fp16, fp32 PSUM accumulation)
  - outT (f16) and denom (f32) ship to DRAM unnormalized; the host computes
    out = (outT / (0.9 * den)).T  (layout choice + cheap elementwise on host)

Precision: fp8 q/k adds ~0.9% relative error through the softmax logits; the
fp16 value path contributes ~5e-4.  Total ~0.9% against the 2e-2 gate.
"""

import sys

for _p in ("/opt/trn_rl_repo",):
    if _p not in sys.path:
        sys.path.append(_p)

import numpy as np

B, S, E, H = 8, 2048, 1024, 128
DROP_P = 0.1
P = 128
W8_SCALE = 16.0

_program_cache = {}


def _build_program(S=S, E=E):
    key = (S, E)
    if key in _program_cache:
        return _program_cache[key]
    NT = S // P   # t-chunks
    NE = E // P   # e-chunks
    SG = 512      # s-group width (one fp32 PSUM bank)
    NSG = S // SG
    NQ = NT // 4  # quads of t-chunks (= s column groups)
    NPAIR = NT // 2  # pairs of t-chunks sharing a 2-bank psum tile

    import concourse.bass as bass  # noqa: F401
    import concourse.mybir as mybir
    import concourse.tile as tile
    from concourse import bacc

    f32 = mybir.dt.float32
    f16 = mybir.dt.float16
    f8 = mybir.dt.float8e4
    Exp = mybir.ActivationFunctionType.Exp
    Copy = mybir.ActivationFunctionType.Copy
    DR = mybir.MatmulPerfMode.DoubleRow
    SCALE = float(E) ** -0.5 / (W8_SCALE * W8_SCALE)

    nc = bacc.Bacc("TRN2", target_bir_lowering=False, debug=False)
    xq_d = nc.dram_tensor("xq", [NQ, P, NE * 4 * P], f16, kind="ExternalInput").ap()
    keep_d = nc.dram_tensor("keepg", [NSG, P, NT * SG], f16, kind="ExternalInput").ap()
    wall_d = nc.dram_tensor("wall", [P, 3 * NE * H], f16, kind="ExternalInput").ap()
    w8_d = nc.dram_tensor("w8", [P, 2 * NE * H], f8, kind="ExternalInput").ap()
    outT_d = nc.dram_tensor("outT", [P, S], f16, kind="ExternalOutput").ap()
    den_d = nc.dram_tensor("den", [1, S], f32, kind="ExternalOutput").ap()

    xq_r = xq_d.rearrange("q p (e b t) -> q p e b t", e=NE, b=4)
    wall_r = wall_d.rearrange("p (j eo h) -> p j eo h", j=3, eo=NE)
    w8_r = w8_d.rearrange("p (j eo h) -> p j eo h", j=2, eo=NE)
    keep_r = keep_d.rearrange("g p (t s) -> g p t s", t=NT)

    with tile.TileContext(nc) as tc:
        with (
            tc.tile_pool(name="consts", bufs=1) as consts,
            tc.tile_pool(name="xw", bufs=1) as xw_pool,
            tc.tile_pool(name="qkv", bufs=1) as qkv_pool,
            tc.tile_pool(name="keep_pool", bufs=8) as keep_pool,
        ):
            ones_t = consts.tile([P, 1], f16)
            nc.vector.memset(ones_t, 1.0)
            warm_sb = consts.tile([P, P], f16)
            nc.vector.memset(warm_sb, 0.0)

            # -------- input DMAs: one issuer (SP), priority order --------
            wall_sb = xw_pool.tile([P, 3, NE, H], f16, tag="wall", name="wall")
            wv_sb = wall_sb[:, 2]
            w8_sb = xw_pool.tile([P, 2, NE, H], f8, tag="w8", name="w8")
            nc.sync.dma_start(w8_sb, w8_r)
            nc.sync.dma_start(wall_sb, wall_r)

            # dropout mask: 4 slices per s-group, fetched on a rolling basis
            keeps = {}  # (sg, sl) -> tile [P, 4, SG]

            def fetch_keep(sg):
                for sl in range(4):
                    kt = keep_pool.tile([P, 4, SG], f16, tag=f"keep{sl}",
                                        name=f"keep{sg}_{sl}")
                    nc.sync.dma_start(kt, keep_r[sg, :, 4 * sl:4 * sl + 4, :])
                    keeps[(sg, sl)] = kt

            # -------- projections, quad-major, DMA-paced --------
            qkT_sb = qkv_pool.tile([P, 2, S], f16)  # [h, (q|k), s]
            v_sb = qkv_pool.tile([P, NT, H], f16)   # v natural: [t_in, t_chunk, h]
            with (
                tc.tile_pool(name="xq_pool", bufs=1) as xq_pool,
                tc.tile_pool(name="proj_ps", bufs=2, space="PSUM") as proj_ps,
            ):
                xq_sbs, x8_sbs = [], []
                for qd in range(NQ):
                    xq_sbs.append(xq_pool.tile([P, NE, 4, P], f16, tag="xq",
                                               name=f"xq{qd}", bufs=3))
                    x8_sbs.append(xq_pool.tile([P, NE, 4, P], f8, tag="x8",
                                               name=f"x8{qd}", bufs=2))
                EH = NE // 2
                for qd in range(NQ):
                    nc.sync.dma_start(xq_sbs[qd][:, 0:EH], xq_r[qd, :, 0:EH])
                    nc.sync.dma_start(xq_sbs[qd][:, EH:NE], xq_r[qd, :, EH:NE])
                fetch_keep(0)

                # PE clock warm-up: keep the array busy during the DMA
                # lead-in so the quad matmuls run at the ramped p-state
                ps_w = proj_ps.tile([P, P], f32, tag="warm", name="ps_warm")
                for i in range(28):
                    nc.tensor.matmul(ps_w, warm_sb, warm_sb,
                                     start=(i == 0), stop=(i == 27))

                for qd in range(NQ):
                    xqt, x8t = xq_sbs[qd], x8_sbs[qd]
                    c_sl = slice(qd * SG, (qd + 1) * SG)
                    # fp8 cast per half-quad on the (otherwise idle) ACT engine
                    nc.scalar.activation(x8t[:, 0:EH], xqt[:, 0:EH], Copy)
                    nc.scalar.activation(x8t[:, EH:NE], xqt[:, EH:NE], Copy)
                    ps_q = proj_ps.tile([P, SG], f32, tag="pq", name="ps_q")
                    ps_k = proj_ps.tile([P, SG], f32, tag="pk", name="ps_k")
                    for ep in range(NE // 2):
                        es = slice(2 * ep, 2 * ep + 2)
                        nc.tensor.matmul(
                            ps_q, w8_sb[:, 0, es, :], x8t[:, es],
                            start=(ep == 0), stop=(ep == NE // 2 - 1),
                            perf_mode=DR,
                        )
                    for ep in range(NE // 2):
                        es = slice(2 * ep, 2 * ep + 2)
                        nc.tensor.matmul(
                            ps_k, w8_sb[:, 1, es, :], x8t[:, es],
                            start=(ep == 0), stop=(ep == NE // 2 - 1),
                            perf_mode=DR,
                        )
                    nc.vector.tensor_copy(qkT_sb[:, 0, c_sl], ps_q)
                    nc.vector.tensor_copy(qkT_sb[:, 1, c_sl], ps_k)
                    for b in range(4):
                        t = 4 * qd + b
                        ps_v = proj_ps.tile([P, H], f32, tag="pv", name="ps_v")
                        for e in range(NE):
                            nc.tensor.matmul(
                                ps_v, xqt[:, e, b, :], wv_sb[:, e, :],
                                start=(e == 0), stop=(e == NE - 1),
                            )
                        nc.vector.tensor_copy(v_sb[:, t, :], ps_v)

            # -------- main attention loop over s-groups --------
            with (
                tc.tile_pool(name="att_ps", bufs=2, space="PSUM") as att_ps,
                tc.tile_pool(name="out_ps", bufs=2, space="PSUM") as out_ps,
                tc.tile_pool(name="den_ps", bufs=2, space="PSUM") as den_ps,
                tc.tile_pool(name="sb", bufs=10) as sb_pool,
                tc.tile_pool(name="sb2", bufs=2) as sb2_pool,
            ):
                # deferred denominator work from the previous s-group:
                # (psum_den, sums2_tile, s_slice)
                pending_den = [None]

                def emit_den_step(j):
                    if pending_den[0] is None or j >= 6:
                        return
                    p_den, p_steps, p_sl = pending_den[0]
                    nc.tensor.matmul(
                        p_den, ones_t, p_steps[j],
                        start=(j == 0), stop=(j == 5),
                    )
                    if j == 5:
                        den_sb = sb2_pool.tile([1, SG], f32, tag="den_sb")
                        nc.scalar.copy(den_sb, p_den)
                        nc.gpsimd.dma_start(den_d[:, p_sl], den_sb)
                        pending_den[0] = None

                for sg in range(NSG):
                    last = sg == NSG - 1
                    s_lo = sg * SG
                    s_sl = slice(s_lo, s_lo + SG)
                    if sg + 1 < NSG:
                        fetch_keep(sg + 1)
                    keep_sls = [keeps.pop((sg, sl)) for sl in range(4)]
                    psum_out = out_ps.tile([P, SG], f32, tag="out")
                    psum_den_t = den_ps.tile([P, SG], f32, tag="den")
                    psum_den = psum_den_t[0:1, :]
                    # adjacent pair-sums (DVE), then second-level sums
                    sums = sb_pool.tile([P, 4, 2, SG], f16, tag="sums",
                                        name=f"sums{sg}", bufs=2)
                    sums2 = sb_pool.tile([P, 2, SG], f16, tag="sums2",
                                         name=f"sums2_{sg}", bufs=2)
                    den_steps = [sums2[:, 0, :], sums2[:, 1, :],
                                 sums[:, 2, 0, :], sums[:, 2, 1, :],
                                 sums[:, 3, 0, :], sums[:, 3, 1, :]]
                    expTs = {}
                    attds = {}

                    def emit_front(j, s_sl=s_sl, keep_sls=keep_sls,
                                   expTs=expTs, attds=attds):
                        psum_att = att_ps.tile([P, 2, SG], f32, tag="att",
                                               name=f"att{j}")
                        for i in range(2):
                            t = 2 * j + i
                            nc.tensor.matmul(
                                psum_att[:, i, :],
                                qkT_sb[:, 1, t * P:(t + 1) * P],  # kT chunk
                                qkT_sb[:, 0, s_sl],               # qT slice
                                start=True,
                                stop=True,
                            )
                        expT = sb_pool.tile([P, 2, SG], f16, tag="exp",
                                            name=f"exp{j}", bufs=4)
                        nc.scalar.activation(expT, psum_att, Exp, scale=SCALE)
                        attd = sb_pool.tile([P, 2, SG], f16, tag="attd",
                                            name=f"attd{j}", bufs=4)
                        ksl = keep_sls[j // 2][:, (2 * j) % 4:(2 * j) % 4 + 2, :]
                        nc.vector.tensor_mul(out=attd, in0=expT, in1=ksl)
                        expTs[j] = expT
                        attds[j] = attd

                    def emit_out(j, psum_out=psum_out, attds=attds):
                        attd = attds.pop(j)
                        for i in range(2):
                            t = 2 * j + i
                            nc.tensor.matmul(
                                psum_out,
                                v_sb[:, t, :],
                                attd[:, i, :],
                                start=(t == 0),
                                stop=(t == NT - 1),
                            )

                    # software pipeline: front(j); the previous group's four
                    # deferred den matmuls on slots 0-3; adjacent pair-sums
                    # after each odd front, level-2 sums after j=3 / j=7;
                    # out(j-1)
                    for j in range(NPAIR):
                        emit_front(j)
                        if not (last and j >= 4):
                            emit_den_step(j)
                        if j % 2 == 1:
                            jp = j // 2
                            nc.vector.tensor_add(
                                out=sums[:, jp],
                                in0=expTs.pop(j - 1), in1=expTs.pop(j),
                            )
                            if jp == 1:
                                nc.vector.tensor_add(
                                    out=sums2, in0=sums[:, 0], in1=sums[:, 1],
                                )
                                if last:
                                    # drain the previous group's den fully,
                                    # then start the final group's den early
                                    emit_den_step(4)
                                    emit_den_step(5)
                                    pending_den[0] = (psum_den, den_steps, s_sl)
                        if last and j >= 4:
                            emit_den_step(j - 4)
                        if j >= 1:
                            emit_out(j - 1)
                    emit_out(NPAIR - 1)
                    if last:
                        emit_den_step(4)
                        emit_den_step(5)
                    else:
                        pending_den[0] = (psum_den, den_steps, s_sl)

                    # unnormalized transposed output -> f16 staging -> DRAM
                    outT_sb = sb2_pool.tile([P, SG], f16, tag="outT")
                    nc.vector.tensor_copy(outT_sb, psum_out)
                    nc.gpsimd.dma_start(outT_d[:, s_sl], outT_sb)

    nc.compile()
    _program_cache[key] = nc
    return nc


def kernel(x, wq, wk, wv, drop_u):
    from concourse import bass_utils

    x = np.asarray(x)
    wq = np.asarray(wq)
    wk = np.asarray(wk)
    wv = np.asarray(wv)
    drop_u = np.asarray(drop_u)

    nc = _build_program()
    in_maps = build_in_maps(x, wq, wk, wv, drop_u)
    last_err = None
    for _attempt in range(3):
        try:
            res = bass_utils.run_bass_kernel_spmd(
                nc, in_maps, core_ids=list(range(B)), trace=False
            )
            break
        except Exception as e:  # transient device errors — retry
            last_err = e
            import time as _time

            _time.sleep(2.0)
    else:
        raise last_err
    out = np.empty((B, S, H), dtype=np.float32)
    for b in range(B):
        outT = res.results[b]["outT"].astype(np.float32)  # [H, S]
        den = np.asarray(res.results[b]["den"]).reshape(1, S).astype(np.float32)
        out[b] = (outT / ((1.0 - DROP_P) * den)).T
    return out


def _chunk_w(w, dtype, scale=1.0):
    NE = E // P
    return np.ascontiguousarray(
        (np.asarray(w) * np.float32(scale))
        .reshape(NE, P, H).transpose(1, 0, 2).reshape(P, NE * H)
    ).astype(dtype)


def build_in_maps(x, wq, wk, wv, drop_u):
    import ml_dtypes

    NT = S // P
    NE = E // P
    SG = 512
    NSG = S // SG
    NQ = NT // 4
    f8 = ml_dtypes.float8_e4m3fn
    wall = np.concatenate(
        [_chunk_w(w, np.float16) for w in (wq, wk, wv)], axis=1
    )  # [P, 3*NE*H]
    w8 = np.concatenate(
        [_chunk_w(w, f8, W8_SCALE) for w in (wq, wk)], axis=1
    )  # [P, 2*NE*H] fp8
    in_maps = []
    for b in range(B):
        # xq[qd][p][(e*4+b')*128+ti] = x[(qd*4+b')*128+ti, e*128+p]
        xq = np.ascontiguousarray(
            np.asarray(x[b]).reshape(NQ, 4, P, NE, P)  # [qd, b', ti, e, p]
            .transpose(0, 4, 3, 1, 2)                  # [qd, p, e, b', ti]
            .reshape(NQ, P, NE * 4 * P)
        ).astype(np.float16)
        keepT = (drop_u[b].T >= np.float32(DROP_P)).astype(np.float16)  # [t, s]
        keep_g = np.ascontiguousarray(
            keepT.reshape(NT, P, NSG, SG).transpose(2, 1, 0, 3).reshape(
                NSG, P, NT * SG
            )
        )
        in_maps.append({"xq": xq, "keepg": keep_g, "wall": wall, "w8": w8})
    return in_maps


# revision 20
# speedup vs baseline: 1.2004x; 1.0393x over previous
"""Single-head attention (B=8, S=2048, E=1024, H=128) with softmax + deterministic
dropout, data-parallel over batch across 8 NeuronCores (one batch element per core).

Per-core layout strategy ("transposed attention", v6):
  - host ships x ONCE, fp16, in quad-major layout xq [4][128][NE*4*128]:
    xq[qd][p][(e*4+b)*128+ti] = x[(qd*4+b)*128+ti, e*128+p], DMA'd in half-quads.
  - ACT casts each half-quad to fp8e4m3 on-chip (the ACT engine is idle during
    the projection phase); q/k projections then run fp8 DoubleRow (2 e-chunks
    per matmul at 0.5 cycles/col) against host-shipped fp8 wq/wk (scaled by 16,
    compensated via softmax scale SCALE/256).  The v projection stays fp16 off
    the same quad tile (value-path precision).  This makes the projection phase
    DMA-paced instead of PE-paced.
  - host ships keep (dropout mask) [NSG, 128, NT*SG] fp16 {0,1} slabs, DMA'd in
    4 slices per s-group.
  - attT[t, s] pairs: two 128x512 fp16 score matmuls into one 2-bank PSUM tile;
    expT = exp(attT * SCALE/256) (ACT, 1024-wide — the attention pacing engine);
    attd = expT * keep (DVE, 1024-wide)
  - denominator: adjacent pair-sums on DVE (exp_2j + exp_2j+1), second-level
    pair-sums on GPSIMD (otherwise idle), then FOUR M=1 512-col PE matmuls per
    s-group, deferred into the next group's slots so they never starve the
    score->exp chain.  The last group keeps its level-2 sums on DVE so the
    drain stays short.
  - outT[h, s] += v[t].T @ attd  (PE fp16, fp32 PSUM accumulation)
  - outT (f16) and denom (f32) ship to DRAM unnormalized; the host computes
    out = (outT / (0.9 * den)).T  (layout choice + cheap elementwise on host)

Precision: fp8 q/k adds ~0.9% relative error through the softmax logits; the
fp16 value path contributes ~5e-4.  Total ~0.9% against the 2e-2 gate.
"""

import sys

for _p in ("/opt/trn_rl_repo",):
    if _p not in sys.path:
        sys.path.append(_p)

import numpy as np

B, S, E, H = 8, 2048, 1024, 128
DROP_P = 0.1
P = 128
W8_SCALE = 16.0

_program_cache = {}


def _build_program(S=S, E=E):
    key = (S, E)
    if key in _program_cache:
        return _program_cache[key]
    NT = S // P   # t-chunks
    NE = E // P   # e-chunks
    SG = 512      # s-group width (one fp32 PSUM bank)
    NSG = S // SG
    NQ = NT // 4  # quads of t-chunks (= s column groups)
    NPAIR = NT // 2  # pairs of t-chunks sharing a 2-bank psum tile

    import concourse.bass as bass  # noqa: F401
    import concourse.mybir as mybir
    import concourse.tile as tile
    from concourse import bacc

    f32 = mybir.dt.float32
    f16 = mybir.dt.float16
    f8 = mybir.dt.float8e4
    Exp = mybir.ActivationFunctionType.Exp
    Copy = mybir.ActivationFunctionType.Copy
    DR = mybir.MatmulPerfMode.DoubleRow
    SCALE = float(E) ** -0.5 / (W8_SCALE * W8_SCALE)

    nc = bacc.Bacc("TRN2", target_bir_lowering=False, debug=False)
    xq_d = nc.dram_tensor("xq", [NQ, P, NE * 4 * P], f16, kind="ExternalInput").ap()
    keep_d = nc.dram_tensor("keepg", [NSG, P, NT * SG], f16, kind="ExternalInput").ap()
    wv_d = nc.dram_tensor("wvc", [P, NE * H], f16, kind="ExternalInput").ap()
    w8_d = nc.dram_tensor("w8", [P, 2 * NE * H], f8, kind="ExternalInput").ap()
    outT_d = nc.dram_tensor("outT", [P, S], f16, kind="ExternalOutput").ap()
    den_d = nc.dram_tensor("den", [1, S], f32, kind="ExternalOutput").ap()

    xq_r = xq_d.rearrange("q p (e b t) -> q p e b t", e=NE, b=4)
    wv_r = wv_d.rearrange("p (eo h) -> p eo h", eo=NE)
    w8_r = w8_d.rearrange("p (j eo h) -> p j eo h", j=2, eo=NE)
    keep_r = keep_d.rearrange("g p (t s) -> g p t s", t=NT)

    with tile.TileContext(nc) as tc:
        with (
            tc.tile_pool(name="consts", bufs=1) as consts,
            tc.tile_pool(name="xw", bufs=1) as xw_pool,
            tc.tile_pool(name="qkv", bufs=1) as qkv_pool,
            tc.tile_pool(name="keep_pool", bufs=8) as keep_pool,
        ):
            ones_t = consts.tile([P, 1], f16)
            nc.vector.memset(ones_t, 1.0)
            warm_sb = consts.tile([P, P], f16)
            nc.vector.memset(warm_sb, 0.0)

            # -------- input DMAs: one issuer (SP), priority order --------
            wv_sb = xw_pool.tile([P, NE, H], f16, tag="wv", name="wv")
            w8_sb = xw_pool.tile([P, 2, NE, H], f8, tag="w8", name="w8")
            nc.sync.dma_start(w8_sb, w8_r)
            nc.sync.dma_start(wv_sb, wv_r)

            # dropout mask: 4 slices per s-group, fetched on a rolling basis
            keeps = {}  # (sg, sl) -> tile [P, 4, SG]

            def fetch_keep(sg):
                for sl in range(4):
                    kt = keep_pool.tile([P, 4, SG], f16, tag=f"keep{sl}",
                                        name=f"keep{sg}_{sl}")
                    nc.sync.dma_start(kt, keep_r[sg, :, 4 * sl:4 * sl + 4, :])
                    keeps[(sg, sl)] = kt

            # -------- projections, quad-major, DMA-paced --------
            qkT_sb = qkv_pool.tile([P, 2, S], f16)  # [h, (q|k), s]
            v_sb = qkv_pool.tile([P, NT, H], f16)   # v natural: [t_in, t_chunk, h]
            with (
                tc.tile_pool(name="xq_pool", bufs=1) as xq_pool,
                tc.tile_pool(name="proj_ps", bufs=2, space="PSUM") as proj_ps,
            ):
                xq_sbs, x8_sbs = [], []
                for qd in range(NQ):
                    xq_sbs.append(xq_pool.tile([P, NE, 4, P], f16, tag="xq",
                                               name=f"xq{qd}", bufs=3))
                    x8_sbs.append(xq_pool.tile([P, NE, 4, P], f8, tag="x8",
                                               name=f"x8{qd}", bufs=2))
                EH = NE // 2
                for qd in range(NQ):
                    nc.sync.dma_start(xq_sbs[qd][:, 0:EH], xq_r[qd, :, 0:EH])
                    nc.sync.dma_start(xq_sbs[qd][:, EH:NE], xq_r[qd, :, EH:NE])
                fetch_keep(0)

                # PE clock warm-up: keep the array busy during the DMA
                # lead-in so the quad matmuls run at the ramped p-state
                ps_w = proj_ps.tile([P, P], f32, tag="warm", name="ps_warm")
                for i in range(52):
                    nc.tensor.matmul(ps_w, warm_sb, warm_sb,
                                     start=(i == 0), stop=(i == 51))

                for qd in range(NQ):
                    xqt, x8t = xq_sbs[qd], x8_sbs[qd]
                    c_sl = slice(qd * SG, (qd + 1) * SG)
                    # fp8 cast per half-quad on the (otherwise idle) ACT engine
                    nc.scalar.activation(x8t[:, 0:EH], xqt[:, 0:EH], Copy)
                    nc.scalar.activation(x8t[:, EH:NE], xqt[:, EH:NE], Copy)
                    ps_q = proj_ps.tile([P, SG], f32, tag="pq", name="ps_q")
                    ps_k = proj_ps.tile([P, SG], f32, tag="pk", name="ps_k")
                    for ep in range(NE // 2):
                        es = slice(2 * ep, 2 * ep + 2)
                        nc.tensor.matmul(
                            ps_q, w8_sb[:, 0, es, :], x8t[:, es],
                            start=(ep == 0), stop=(ep == NE // 2 - 1),
                            perf_mode=DR,
                        )
                    for ep in range(NE // 2):
                        es = slice(2 * ep, 2 * ep + 2)
                        nc.tensor.matmul(
                            ps_k, w8_sb[:, 1, es, :], x8t[:, es],
                            start=(ep == 0), stop=(ep == NE // 2 - 1),
                            perf_mode=DR,
                        )
                    nc.vector.tensor_copy(qkT_sb[:, 0, c_sl], ps_q)
                    nc.vector.tensor_copy(qkT_sb[:, 1, c_sl], ps_k)
                    for b in range(4):
                        t = 4 * qd + b
                        ps_v = proj_ps.tile([P, H], f32, tag="pv", name="ps_v")
                        for e in range(NE):
                            nc.tensor.matmul(
                                ps_v, xqt[:, e, b, :], wv_sb[:, e, :],
                                start=(e == 0), stop=(e == NE - 1),
                            )
                        nc.vector.tensor_copy(v_sb[:, t, :], ps_v)

            # -------- main attention loop over s-groups --------
            with (
                tc.tile_pool(name="att_ps", bufs=2, space="PSUM") as att_ps,
                tc.tile_pool(name="out_ps", bufs=2, space="PSUM") as out_ps,
                tc.tile_pool(name="den_ps", bufs=2, space="PSUM") as den_ps,
                tc.tile_pool(name="sb", bufs=10) as sb_pool,
                tc.tile_pool(name="sb2", bufs=2) as sb2_pool,
            ):
                # deferred denominator work from the previous s-group:
                # (psum_den, sums2_tile, s_slice)
                pending_den = [None]

                def emit_den_step(j):
                    if pending_den[0] is None or j >= 6:
                        return
                    p_den, p_steps, p_sl = pending_den[0]
                    nc.tensor.matmul(
                        p_den, ones_t, p_steps[j],
                        start=(j == 0), stop=(j == 5),
                    )
                    if j == 5:
                        den_sb = sb2_pool.tile([1, SG], f32, tag="den_sb")
                        nc.scalar.copy(den_sb, p_den)
                        nc.sync.dma_start(den_d[:, p_sl], den_sb)
                        pending_den[0] = None

                for sg in range(NSG):
                    last = sg == NSG - 1
                    s_lo = sg * SG
                    s_sl = slice(s_lo, s_lo + SG)
                    if sg + 1 < NSG:
                        fetch_keep(sg + 1)
                    keep_sls = [keeps.pop((sg, sl)) for sl in range(4)]
                    psum_out = out_ps.tile([P, SG], f32, tag="out")
                    psum_den_t = den_ps.tile([P, SG], f32, tag="den")
                    psum_den = psum_den_t[0:1, :]
                    # adjacent pair-sums (DVE), then second-level sums
                    sums = sb_pool.tile([P, 4, 2, SG], f16, tag="sums",
                                        name=f"sums{sg}", bufs=2)
                    sums2 = sb_pool.tile([P, 2, SG], f16, tag="sums2",
                                         name=f"sums2_{sg}", bufs=2)
                    den_steps = [sums2[:, 0, :], sums2[:, 1, :],
                                 sums[:, 2, 0, :], sums[:, 2, 1, :],
                                 sums[:, 3, 0, :], sums[:, 3, 1, :]]
                    expTs = {}
                    attds = {}

                    def emit_front(j, s_sl=s_sl, keep_sls=keep_sls,
                                   expTs=expTs, attds=attds):
                        psum_att = att_ps.tile([P, 2, SG], f32, tag="att",
                                               name=f"att{j}")
                        for i in range(2):
                            t = 2 * j + i
                            nc.tensor.matmul(
                                psum_att[:, i, :],
                                qkT_sb[:, 1, t * P:(t + 1) * P],  # kT chunk
                                qkT_sb[:, 0, s_sl],               # qT slice
                                start=True,
                                stop=True,
                            )
                        expT = sb_pool.tile([P, 2, SG], f16, tag="exp",
                                            name=f"exp{j}", bufs=6)
                        nc.scalar.activation(expT, psum_att, Exp, scale=SCALE)
                        attd = sb_pool.tile([P, 2, SG], f16, tag="attd",
                                            name=f"attd{j}", bufs=6)
                        ksl = keep_sls[j // 2][:, (2 * j) % 4:(2 * j) % 4 + 2, :]
                        nc.vector.tensor_mul(out=attd, in0=expT, in1=ksl)
                        expTs[j] = expT
                        attds[j] = attd

                    def emit_out(j, psum_out=psum_out, attds=attds):
                        attd = attds.pop(j)
                        for i in range(2):
                            t = 2 * j + i
                            nc.tensor.matmul(
                                psum_out,
                                v_sb[:, t, :],
                                attd[:, i, :],
                                start=(t == 0),
                                stop=(t == NT - 1),
                            )

                    # software pipeline: front(j); the previous group's four
                    # deferred den matmuls on slots 0-3; adjacent pair-sums
                    # after each odd front, level-2 sums after j=3 / j=7;
                    # out(j-1)
                    for j in range(NPAIR):
                        emit_front(j)
                        if not (last and j >= 4):
                            emit_den_step(j)
                        if j % 2 == 1:
                            jp = j // 2
                            nc.vector.tensor_add(
                                out=sums[:, jp],
                                in0=expTs.pop(j - 1), in1=expTs.pop(j),
                            )
                            if jp == 1:
                                nc.vector.tensor_add(
                                    out=sums2, in0=sums[:, 0], in1=sums[:, 1],
                                )
                                if last:
                                    # drain the previous group's den fully,
                                    # then start the final group's den early
                                    emit_den_step(4)
                                    emit_den_step(5)
                                    pending_den[0] = (psum_den, den_steps, s_sl)
                        if last and j >= 4:
                            emit_den_step(j - 4)
                        if j >= 1:
                            emit_out(j - 1)
                    emit_out(NPAIR - 1)
                    if last:
                        emit_den_step(4)
                        emit_den_step(5)
                    else:
                        pending_den[0] = (psum_den, den_steps, s_sl)

                    # unnormalized transposed output -> f16 staging -> DRAM
                    outT_sb = sb2_pool.tile([P, SG], f16, tag="outT")
                    nc.vector.tensor_copy(outT_sb, psum_out)
                    nc.sync.dma_start(outT_d[:, s_sl], outT_sb)

    nc.compile()
    _program_cache[key] = nc
    return nc


def kernel(x, wq, wk, wv, drop_u):
    from concourse import bass_utils

    x = np.asarray(x)
    wq = np.asarray(wq)
    wk = np.asarray(wk)
    wv = np.asarray(wv)
    drop_u = np.asarray(drop_u)

    nc = _build_program()
    in_maps = build_in_maps(x, wq, wk, wv, drop_u)
    last_err = None
    for _attempt in range(3):
        try:
            res = bass_utils.run_bass_kernel_spmd(
                nc, in_maps, core_ids=list(range(B)), trace=False
            )
            break
        except Exception as e:  # transient device errors — retry
            last_err = e
            import time as _time

            _time.sleep(2.0)
    else:
        raise last_err
    out = np.empty((B, S, H), dtype=np.float32)
    for b in range(B):
        outT = res.results[b]["outT"].astype(np.float32)  # [H, S]
        den = np.asarray(res.results[b]["den"]).reshape(1, S).astype(np.float32)
        out[b] = (outT / ((1.0 - DROP_P) * den)).T
    return out


def _chunk_w(w, dtype, scale=1.0):
    NE = E // P
    return np.ascontiguousarray(
        (np.asarray(w) * np.float32(scale))
        .reshape(NE, P, H).transpose(1, 0, 2).reshape(P, NE * H)
    ).astype(dtype)


def build_in_maps(x, wq, wk, wv, drop_u):
    import ml_dtypes

    NT = S // P
    NE = E // P
    SG = 512
    NSG = S // SG
    NQ = NT // 4
    f8 = ml_dtypes.float8_e4m3fn
    wvc = _chunk_w(wv, np.float16)  # [P, NE*H]
    w8 = np.concatenate(
        [_chunk_w(w, f8, W8_SCALE) for w in (wq, wk)], axis=1
    )  # [P, 2*NE*H] fp8
    in_maps = []
    for b in range(B):
        # xq[qd][p][(e*4+b')*128+ti] = x[(qd*4+b')*128+ti, e*128+p]
        xq = np.ascontiguousarray(
            np.asarray(x[b]).reshape(NQ, 4, P, NE, P)  # [qd, b', ti, e, p]
            .transpose(0, 4, 3, 1, 2)                  # [qd, p, e, b', ti]
            .reshape(NQ, P, NE * 4 * P)
        ).astype(np.float16)
        keepT = (drop_u[b].T >= np.float32(DROP_P)).astype(np.float16)  # [t, s]
        keep_g = np.ascontiguousarray(
            keepT.reshape(NT, P, NSG, SG).transpose(2, 1, 0, 3).reshape(
                NSG, P, NT * SG
            )
        )
        in_maps.append({"xq": xq, "keepg": keep_g, "wvc": wvc, "w8": w8})
    return in_maps
